# revision 1
# baseline (speedup 1.0000x reference)
"""GQA attention kernel for Trainium2, 8 NeuronCores.

Problem: resid [2, 2048, 1024], 16 Q heads / 8 KV groups, d_head 64, causal,
out = softmax(QK^T/8 + causal) V -> W_out + b_out.

Sharding: tensor-parallel over (batch x kv-group-pairs). Core c handles
batch b = c // 4 and kv groups {2*(c%4), 2*(c%4)+1} = 4 Q heads. Each core
computes its heads' attention and a partial output projection; the host sums
the 4 partials per batch element and adds b_out.

Per-core dataflow (fp32 storage, float32r matmuls = full PE speed at
moving-dim >= 256):
  - host passes resid[b].T so the d_model contraction lands on partitions
  - Q^T [256, S] and K^T [128, S] projections (PSUM accum over 8 d-chunks)
  - V [S, 2x65] with a ones column appended per group -> the AV matmul
    produces sum-exp for free in output row 64
  - scores computed transposed: S^T[k, q] = K @ Q^T; causality via q-start
    offset, zero-padding of exp tiles, and an upper-triangular
    multiplicative mask on diagonal tiles
  - softmax without max-subtraction (scores are O(1) by construction;
    masked lanes are exactly zero after the mask multiply)
  - U^T[e, q] += V_aug^T @ exp accumulated over k-tiles in PSUM
  - normalize: reciprocal of row 64 (VectorE), partition-broadcast
    (GpSimd), multiply into z^T (VectorE)
  - out_partial[s, d] = z^T.T @ W_out_stack accumulated over 2 e-chunks
"""

import sys

sys.path.insert(0, "/opt/trn_rl_repo")

import numpy as np

import concourse.bass as bass
import concourse.mybir as mybir
import concourse.tile as tile
from concourse import bacc
from concourse.bass_utils import run_bass_kernel_spmd
from concourse.masks import make_upper_triangular

S = 2048          # seq len
D = 1024          # d_model
E = 64            # d_head
P = 128
NC_HEADS = 4      # heads per core
NCHUNK = D // P   # 8 d_model chunks
SPAN = 512
NSPAN = S // SPAN
NKT = S // P      # 16 k tiles
F32 = mybir.dt.float32
F32R = mybir.dt.float32r
EXP = mybir.ActivationFunctionType.Exp

LAST_RESULTS = None  # stashed BassKernelResults for the test harness
_CACHED_NC = None


def _build_program():
    nc = bacc.Bacc("TRN2", target_bir_lowering=False, debug=False)

    rT_d = nc.dram_tensor("resid_t", [D, S], F32R, kind="ExternalInput")
    wq_d = nc.dram_tensor("wq", [D, 256], F32R, kind="ExternalInput")
    wk_d = nc.dram_tensor("wk", [D, 128], F32R, kind="ExternalInput")
    wv_d = nc.dram_tensor("wv", [D, 128], F32R, kind="ExternalInput")
    wo_d = nc.dram_tensor("wo", [256, D], F32R, kind="ExternalInput")
    ones_d = nc.dram_tensor("ones", [P, 1], F32R, kind="ExternalInput")
    out_d = nc.dram_tensor("out", [S, D], F32, kind="ExternalOutput")

    with tile.TileContext(nc) as tc:
        with (
            tc.tile_pool(name="persist", bufs=1) as pp,
            tc.tile_pool(name="exp", bufs=12) as ep,
            tc.tile_pool(name="zt", bufs=3) as zp,
            tc.tile_pool(name="misc", bufs=4) as mp,
            tc.tile_pool(name="ostage", bufs=4) as op,
            tc.tile_pool(name="ps_u", bufs=4, space="PSUM") as ps_u,
            tc.tile_pool(name="ps_sc", bufs=3, space="PSUM") as ps_sc,
            tc.tile_pool(name="ps_op", bufs=1, space="PSUM") as ps_op,
        ):
            # ---- load weights + transposed residual ----
            wq_sb = []
            wk_sb = []
            wv_sb = []
            for c in range(NCHUNK):
                t = pp.tile([P, 256], F32R, tag=f"wq{c}")
                nc.sync.dma_start(t[:], wq_d[c * P:(c + 1) * P, :])
                wq_sb.append(t)
                t = pp.tile([P, 128], F32R, tag=f"wk{c}")
                nc.sync.dma_start(t[:], wk_d[c * P:(c + 1) * P, :])
                wk_sb.append(t)
                t = pp.tile([P, 128], F32R, tag=f"wv{c}")
                nc.sync.dma_start(t[:], wv_d[c * P:(c + 1) * P, :])
                wv_sb.append(t)
            wo_sb = []
            for c in range(2):
                t = pp.tile([P, D], F32R, tag=f"wo{c}")
                nc.sync.dma_start(t[:], wo_d[c * P:(c + 1) * P, :])
                wo_sb.append(t)

            mask = pp.tile([P, P], F32, tag="mask")
            make_upper_triangular(nc, mask[:], val=1.0, diag=True)

            # residual chunks, DMA'd span-wise so projection accumulation
            # groups (which need all 8 d-chunks of one span) start after
            # ~2MB instead of the full 8.4MB
            rT = []
            for c in range(NCHUNK):
                t = pp.tile([P, S], F32R, tag=f"rt{c}", name=f"rt{c}")
                rT.append(t)
            for sp in range(NSPAN):
                for c in range(NCHUNK):
                    nc.sync.dma_start(
                        rT[c][:, sp * SPAN:(sp + 1) * SPAN],
                        rT_d[c * P:(c + 1) * P, sp * SPAN:(sp + 1) * SPAN])

            qT = [pp.tile([P, S], F32R, tag=f"qt{e}", name=f"qt{e}")
                  for e in range(2)]
            kT = pp.tile([P, S], F32R, tag="kt")
            vaug = [pp.tile([P, 130], F32R, tag=f"va{k}", name=f"va{k}")
                    for k in range(NKT)]

            # ---- per span: Q/K/V projection for this span, then attention.
            # Interleaving keeps PE fed with projection matmuls while ACT
            # (the exp bottleneck) works through the previous tiles. ----
            for sp in range(NSPAN):
                for eblk in range(2):
                    acc = ps_u.tile([P, SPAN], F32, tag="u", name="qacc")
                    for c in range(NCHUNK):
                        nc.tensor.matmul(
                            acc[:],
                            wq_sb[c][:, eblk * P:(eblk + 1) * P],
                            rT[c][:, sp * SPAN:(sp + 1) * SPAN],
                            start=(c == 0),
                            stop=(c == NCHUNK - 1),
                        )
                    nc.vector.tensor_copy(
                        qT[eblk][:, sp * SPAN:(sp + 1) * SPAN], acc[:])
                acc = ps_u.tile([P, SPAN], F32, tag="u", name="kacc")
                for c in range(NCHUNK):
                    nc.tensor.matmul(
                        acc[:],
                        wk_sb[c][:],
                        rT[c][:, sp * SPAN:(sp + 1) * SPAN],
                        start=(c == 0),
                        stop=(c == NCHUNK - 1),
                    )
                nc.vector.tensor_copy(kT[:, sp * SPAN:(sp + 1) * SPAN], acc[:])
                for kt in range(4 * sp, 4 * sp + 4):
                    va = vaug[kt]
                    acc = ps_sc.tile([P, SPAN], F32, tag="sc", name="vacc")
                    for c in range(NCHUNK):
                        nc.tensor.matmul(
                            acc[:, 0:128],
                            rT[c][:, kt * P:(kt + 1) * P],
                            wv_sb[c][:],
                            start=(c == 0),
                            stop=(c == NCHUNK - 1),
                        )
                    nc.vector.tensor_copy(va[:, 0:64], acc[:, 0:64])
                    nc.vector.tensor_copy(va[:, 65:129], acc[:, 64:128])
                    nc.sync.dma_start(va[:, 64:65], ones_d[:])
                    nc.sync.dma_start(va[:, 129:130], ones_d[:])

                q0 = sp * SPAN
                nkt = (q0 + SPAN) // P  # k tiles touching this span
                # head slot (g, i): local head 2g+i, stored in qT[i] rows
                # g*64:(g+1)*64 so scores lhsT/rhs share base partition g*64
                # (and g0/g1 matmuls row-pack the PE array).
                u_ps = [ps_u.tile([P, SPAN], F32, tag="u", name=f"u{j}")
                        for j in range(NC_HEADS)]
                # software pipeline: AV of k-tile kt is emitted after the
                # scores+exp of kt+1, hiding the ACT exp latency from PE
                def emit_av(batch):
                    for g, i, e_sb, kt_, off_, w_ in batch:
                        nc.tensor.matmul(
                            u_ps[2 * g + i][0:65, off_:off_ + w_],
                            vaug[kt_][:, g * 65:(g + 1) * 65],
                            e_sb[:, off_:off_ + w_],
                            start=(kt_ == 0),
                            stop=(kt_ == nkt - 1),
                            skip_group_check=True,
                        )

                pending = []
                for kt in range(nkt):
                    k0 = kt * P
                    off = max(k0 - q0, 0)
                    w = SPAN - off
                    cur = []
                    for g in range(2):
                        for i in range(2):
                            s_ps = ps_sc.tile([P, SPAN], F32, tag="sc",
                                              name=f"s{g}{i}")
                            nc.tensor.matmul(
                                s_ps[:, off:off + w],
                                kT[g * 64:(g + 1) * 64, k0:k0 + P],
                                qT[i][g * 64:(g + 1) * 64,
                                         q0 + off:q0 + off + w],
                                start=True,
                                stop=True,
                            )
                            e_sb = ep.tile([P, SPAN], F32R, tag="e",
                                           name=f"e{g}{i}")
                            nc.scalar.activation(
                                e_sb[:, off:off + w], s_ps[:, off:off + w],
                                EXP, scale=0.125,
                            )
                            if k0 >= q0:  # diagonal tile -> causal mask
                                nc.vector.tensor_mul(
                                    e_sb[:, off:off + P],
                                    e_sb[:, off:off + P].bitcast(F32),
                                    mask[:],
                                )
                            cur.append((g, i, e_sb, kt, off, w))
                    emit_av(pending)
                    pending = cur
                emit_av(pending)

                # normalize -> z^T chunks; zc[i] rows g*64 = head slot (g, i),
                # matching the host-side wo packing [h0, h2 | h1, h3]
                zc = [zp.tile([P, SPAN], F32R, tag=f"zt{c}", name=f"z{c}")
                      for c in range(2)]
                for g in range(2):
                    for i in range(2):
                        # 1/x as exp(-ln x) on ScalarE: ~4x faster than the
                        # single-partition DVE reciprocal and off its queue
                        lnt = mp.tile([1, SPAN], F32, tag="ln", name="lnt")
                        nc.scalar.activation(
                            lnt[:], u_ps[2 * g + i][64:65, :],
                            mybir.ActivationFunctionType.Ln)
                        rec = mp.tile([1, SPAN], F32, tag="rec", name="rec")
                        nc.scalar.activation(rec[:], lnt[:], EXP, scale=-1.0)
                        bc = mp.tile([64, SPAN], F32, tag="bc", name="bc")
                        nc.gpsimd.partition_broadcast(bc[:], rec[:])
                        nc.vector.tensor_mul(
                            zc[i][g * 64:(g + 1) * 64, :],
                            u_ps[2 * g + i][0:64, :],
                            bc[:],
                        )

                # output projection for this span of s
                for st in range(4):
                    s0 = q0 + st * P
                    o_sb = op.tile([P, D], F32, tag="ost")
                    for dsp in range(2):
                        o_ps = ps_op.tile([P, SPAN], F32, tag="op")
                        for ch in range(2):
                            nc.tensor.matmul(
                                o_ps[:],
                                zc[ch][:, st * P:(st + 1) * P],
                                wo_sb[ch][:, dsp * SPAN:(dsp + 1) * SPAN],
                                start=(ch == 0),
                                stop=(ch == 1),
                            )
                        nc.vector.tensor_copy(
                            o_sb[:, dsp * SPAN:(dsp + 1) * SPAN], o_ps[:])
                    nc.sync.dma_start(out_d[s0:s0 + P, :], o_sb[:])

    nc.finalize()
    return nc


def kernel(resid, W_Q, W_K, W_V, W_out, b_out):
    global LAST_RESULTS, _CACHED_NC
    resid = np.asarray(resid, np.float32)
    W_Q = np.asarray(W_Q, np.float32)
    W_K = np.asarray(W_K, np.float32)
    W_V = np.asarray(W_V, np.float32)
    W_out = np.asarray(W_out, np.float32)
    b_out = np.asarray(b_out, np.float32)

    if _CACHED_NC is None:
        _CACHED_NC = _build_program()
    nc = _CACHED_NC

    residT = [np.ascontiguousarray(resid[b].T) for b in range(2)]
    in_maps = []
    for c in range(8):
        b, q = c // 4, c % 4
        # interleaved head order [h0, h2, h1, h3]: storage slot (g, i) holds
        # local head 2g+i -> qT[i]/zc[i] rows g*64 (see _build_program)
        heads = [4 * q, 4 * q + 2, 4 * q + 1, 4 * q + 3]
        groups = [2 * q, 2 * q + 1]
        in_maps.append({
            "resid_t": residT[b],
            "wq": np.ascontiguousarray(W_Q[:, heads, :].reshape(D, 256)),
            "wk": np.ascontiguousarray(W_K[:, groups, :].reshape(D, 128)),
            "wv": np.ascontiguousarray(W_V[:, groups, :].reshape(D, 128)),
            "wo": np.ascontiguousarray(
                W_out[:, heads, :].transpose(1, 0, 2).reshape(256, D)),
            "ones": np.ones((P, 1), np.float32),
        })

    res = run_bass_kernel_spmd(nc, in_maps, core_ids=list(range(8)))
    LAST_RESULTS = res

    out = np.zeros((2, S, D), np.float32)
    for c in range(8):
        out[c // 4] += res.results[c]["out"]
    out += b_out
    return out



# revision 2
# speedup vs baseline: 1.2675x; 1.2675x over previous
"""GQA attention kernel for Trainium2, 8 NeuronCores.

Problem: resid [2, 2048, 1024], 16 Q heads / 8 KV groups, d_head 64, causal,
out = softmax(QK^T/8 + causal) V -> W_out + b_out.

Sharding: tensor-parallel over (batch x kv-group-pairs). Core c handles
batch b = c // 4 and kv groups {2*(c%4), 2*(c%4)+1} = 4 Q heads. Each core
computes its heads' attention and a partial output projection; the host sums
the 4 partials per batch element and adds b_out.

Per-core dataflow (bf16 storage/matmuls, fp32 PSUM accumulation):
  - host passes resid[b].T in bf16 so the d_model contraction lands on
    partitions
  - Q^T [256, S] and K^T [128, S] projections (PSUM accum over 8 d-chunks)
  - V [S, 2x65] with a ones column appended per group -> the AV matmul
    produces sum-exp for free in output row 64
  - scores computed transposed: S^T[k, q] = K @ Q^T; causality via q-start
    offset, zero-padding of exp tiles, and an upper-triangular
    multiplicative mask on diagonal tiles
  - per k-tile the two group-pairs' scores land in one 2-bank PSUM tile so
    a single ACT exp instruction (2D access pattern) covers both heads
  - softmax without max-subtraction (scores are O(1) by construction;
    masked lanes are exactly zero after the mask multiply)
  - U^T[e, q] += V_aug^T @ exp accumulated over k-tiles into one 4-bank
    PSUM tile (one 512-col slot per head) so the whole span's sum-exp row
    is one contiguous [1, 2048] AP: one Ln + one Exp(-x) per span
  - a pre-placed ACT table load of natural_log_exp_and_others serves both
    Exp and Ln (the default pass thrashes 17 table loads otherwise)
  - normalize: partition-broadcast of 1/sumexp (GpSimd), multiply (DVE)
  - out_partial[s, d] = z^T.T @ W_out_stack accumulated over 2 e-chunks
  - next span's Q/K/V projection matmul groups are statically interleaved
    into the attention k-tile stream so the PE stays fed while ACT exps
"""

import sys

sys.path.insert(0, "/opt/trn_rl_repo")

import ml_dtypes
import numpy as np

import concourse.bass as bass
import concourse.mybir as mybir
import concourse.tile as tile
from concourse import bacc
from concourse.bass_utils import run_bass_kernel_spmd
from concourse.hw_specs import get_activation_tables
from concourse.masks import make_upper_triangular

S = 2048          # seq len
D = 1024          # d_model
E = 64            # d_head
P = 128
NC_HEADS = 4      # heads per core
NCHUNK = D // P   # 8 d_model chunks
SPAN = 512
NSPAN = S // SPAN
NKT = S // P      # 16 k tiles
F32 = mybir.dt.float32
BF = mybir.dt.bfloat16
EXP = mybir.ActivationFunctionType.Exp
LN = mybir.ActivationFunctionType.Ln

LAST_RESULTS = None  # stashed BassKernelResults for the test harness
_CACHED_NC = None


def _build_program():
    nc = bacc.Bacc("TRN2", target_bir_lowering=False, debug=False)

    rT_d = nc.dram_tensor("resid_t", [D, S], BF, kind="ExternalInput")
    wq_d = nc.dram_tensor("wq", [D, 256], BF, kind="ExternalInput")
    wk_d = nc.dram_tensor("wk", [D, 128], BF, kind="ExternalInput")
    wv_d = nc.dram_tensor("wv", [D, 128], BF, kind="ExternalInput")
    wo_d = nc.dram_tensor("wo", [256, D], BF, kind="ExternalInput")
    out_d = nc.dram_tensor("out", [S, D], F32, kind="ExternalOutput")

    # one ACT table set serves Exp and Ln; pre-placing the load keeps the
    # fixpoint pass from alternating exp_and_others / natural_log sets
    tables = list(get_activation_tables(nc.m.arch).keys())
    nle_id = tables.index("natural_log_exp_and_others")

    with tile.TileContext(nc) as tc:
        with (
            tc.tile_pool(name="persist", bufs=1) as pp,
            tc.tile_pool(name="exp", bufs=6) as ep,
            tc.tile_pool(name="zt", bufs=2) as zp,
            tc.tile_pool(name="misc", bufs=2) as mp,
            tc.tile_pool(name="ostage", bufs=3) as op,
            tc.tile_pool(name="ps_u", bufs=1, space="PSUM") as ps_u,
            tc.tile_pool(name="ps_sc", bufs=2, space="PSUM") as ps_sc,
        ):
            nc.scalar.add_instruction(
                mybir.InstLoadActFuncSet(
                    name=nc.get_next_instruction_name(),
                    act_func_set_id=nle_id,
                )
            )

            # ---- load weights + transposed residual ----
            wq_sb = []
            wk_sb = []
            wv_sb = []
            for c in range(NCHUNK):
                t = pp.tile([P, 256], BF, tag=f"wq{c}", name=f"wq{c}")
                nc.sync.dma_start(t[:], wq_d[c * P:(c + 1) * P, :])
                wq_sb.append(t)
                t = pp.tile([P, 128], BF, tag=f"wk{c}", name=f"wk{c}")
                nc.sync.dma_start(t[:], wk_d[c * P:(c + 1) * P, :])
                wk_sb.append(t)
                t = pp.tile([P, 128], BF, tag=f"wv{c}", name=f"wv{c}")
                nc.sync.dma_start(t[:], wv_d[c * P:(c + 1) * P, :])
                wv_sb.append(t)
            wo_sb = []
            for c in range(2):
                t = pp.tile([P, D], BF, tag=f"wo{c}", name=f"wo{c}")
                nc.sync.dma_start(t[:], wo_d[c * P:(c + 1) * P, :])
                wo_sb.append(t)

            mask = pp.tile([P, P], BF, tag="mask")
            make_upper_triangular(nc, mask[:], val=1.0, diag=True)

            # residual chunks, DMA'd span-wise so projection accumulation
            # groups (which need all 8 d-chunks of one span) start early
            rT = []
            for c in range(NCHUNK):
                t = pp.tile([P, S], BF, tag=f"rt{c}", name=f"rt{c}")
                rT.append(t)
            for sp in range(NSPAN):
                for c in range(NCHUNK):
                    nc.sync.dma_start(
                        rT[c][:, sp * SPAN:(sp + 1) * SPAN],
                        rT_d[c * P:(c + 1) * P, sp * SPAN:(sp + 1) * SPAN])

            qT = [pp.tile([P, S], BF, tag=f"qt{e}", name=f"qt{e}")
                  for e in range(2)]
            kT = pp.tile([P, S], BF, tag="kt")
            vaug = [pp.tile([P, 130], BF, tag=f"va{k}", name=f"va{k}")
                    for k in range(NKT)]
            for k in range(NKT):
                nc.gpsimd.memset(vaug[k][:, 64:65], 1.0)
                nc.gpsimd.memset(vaug[k][:, 129:130], 1.0)

            # ---- projection-group emitters (used as fillers inside the
            # attention k-tile loop of the previous span) ----
            def q_proj(sp, eblk):
                acc = ps_sc.tile([P, SPAN], F32, tag="sc", name="qacc")
                for c in range(NCHUNK):
                    nc.tensor.matmul(
                        acc[:],
                        wq_sb[c][:, eblk * P:(eblk + 1) * P],
                        rT[c][:, sp * SPAN:(sp + 1) * SPAN],
                        start=(c == 0),
                        stop=(c == NCHUNK - 1),
                    )
                nc.vector.tensor_copy(
                    qT[eblk][:, sp * SPAN:(sp + 1) * SPAN], acc[:])

            def k_proj(sp):
                acc = ps_sc.tile([P, SPAN], F32, tag="sc", name="kacc")
                for c in range(NCHUNK):
                    nc.tensor.matmul(
                        acc[:],
                        wk_sb[c][:],
                        rT[c][:, sp * SPAN:(sp + 1) * SPAN],
                        start=(c == 0),
                        stop=(c == NCHUNK - 1),
                    )
                nc.vector.tensor_copy(kT[:, sp * SPAN:(sp + 1) * SPAN], acc[:])

            def v_proj(kt):
                va = vaug[kt]
                acc = ps_sc.tile([P, 128], F32, tag="sc", name="vacc")
                for c in range(NCHUNK):
                    nc.tensor.matmul(
                        acc[:, 0:128],
                        rT[c][:, kt * P:(kt + 1) * P],
                        wv_sb[c][:],
                        start=(c == 0),
                        stop=(c == NCHUNK - 1),
                    )
                nc.vector.tensor_copy(va[:, 0:64], acc[:, 0:64])
                nc.vector.tensor_copy(va[:, 65:129], acc[:, 64:128])

            def proj_thunks(sp):
                th = [lambda e=e: q_proj(sp, e) for e in range(2)]
                th.append(lambda: k_proj(sp))
                th += [lambda kt=kt: v_proj(kt)
                       for kt in range(4 * sp, 4 * sp + 4)]
                return th

            # span 0 projections run upfront (overlap the resid DMA tail)
            for th in proj_thunks(0):
                th()

            # ---- per span: attention, with next span's projections woven
            # into the k-tile stream so the PE stays busy while ACT exps ----
            for sp in range(NSPAN):
                fillers = proj_thunks(sp + 1) if sp + 1 < NSPAN else []

                q0 = sp * SPAN
                nkt = (q0 + SPAN) // P  # k tiles touching this span
                # u_big col layout: head slot s = 2g+i at cols s*512;
                # row 64 of each slot accumulates sum-exp (ones column)
                u_big = ps_u.tile([P, 4 * SPAN], F32, tag="u", name="ubig")

                # software pipeline: AV of k-tile kt is emitted after the
                # scores+exp of kt+1, hiding the ACT exp latency from PE
                def emit_av(batch):
                    for g, i, e_sb, kt_, off_, w_ in batch:
                        s_slot = 2 * g + i
                        nc.tensor.matmul(
                            u_big[0:65,
                                  s_slot * SPAN + off_:
                                  s_slot * SPAN + off_ + w_],
                            vaug[kt_][:, g * 65:(g + 1) * 65],
                            e_sb[:, i * SPAN + off_:i * SPAN + off_ + w_],
                            start=(kt_ == 0),
                            stop=(kt_ == nkt - 1),
                            skip_group_check=True,
                        )

                pending = []
                for kt in range(nkt):
                    k0 = kt * P
                    off = max(k0 - q0, 0)
                    w = SPAN - off
                    cur = []
                    for g in range(2):
                        # both i-heads of group g share one 2-bank PSUM
                        # tile -> a single exp instruction covers them
                        pair = ps_sc.tile([P, 2 * SPAN], F32, tag="sc",
                                          name=f"pair{g}")
                        for i in range(2):
                            nc.tensor.matmul(
                                pair[:, i * SPAN + off:i * SPAN + off + w],
                                kT[g * 64:(g + 1) * 64, k0:k0 + P],
                                qT[i][g * 64:(g + 1) * 64,
                                      q0 + off:q0 + off + w],
                                start=True,
                                stop=True,
                            )
                        e_sb = ep.tile([P, 2 * SPAN], BF, tag="e",
                                       name=f"e{g}")
                        pv = pair.rearrange("p (i w) -> p i w", i=2)
                        ev = e_sb.rearrange("p (i w) -> p i w", i=2)
                        nc.scalar.activation(
                            ev[:, :, off:off + w], pv[:, :, off:off + w],
                            EXP, scale=0.125,
                        )
                        if k0 >= q0:  # diagonal tile -> causal mask
                            for i in range(2):
                                nc.vector.tensor_mul(
                                    e_sb[:, i * SPAN + off:i * SPAN + off + P],
                                    e_sb[:, i * SPAN + off:i * SPAN + off + P],
                                    mask[:],
                                )
                        cur.append((g, 0, e_sb, kt, off, w))
                        cur.append((g, 1, e_sb, kt, off, w))
                    emit_av(pending)
                    pending = cur
                    if fillers:
                        fillers.pop(0)()
                emit_av(pending)
                for th in fillers:
                    th()

                # ---- normalize -> z^T chunks; zc[i] rows g*64 = head slot
                # (g, i), matching the host-side wo packing [h0,h2|h1,h3].
                # One Ln + one Exp(-x) per span over the contiguous [1, 2048]
                # sum-exp row (row 64 across the 4 u slots). ----
                lnt = mp.tile([1, 4 * SPAN], F32, tag="ln", name="lnt")
                nc.scalar.activation(lnt[:], u_big[64:65, :], LN)
                rec = mp.tile([1, 4 * SPAN], F32, tag="rec", name="rec")
                nc.scalar.activation(rec[:], lnt[:], EXP, scale=-1.0)
                zc = [zp.tile([P, SPAN], BF, tag=f"zt{c}", name=f"z{c}")
                      for c in range(2)]
                for g in range(2):
                    for i in range(2):
                        s_slot = 2 * g + i
                        bc = mp.tile([64, SPAN], F32, tag=f"bc{s_slot}",
                                     name="bc")
                        nc.gpsimd.partition_broadcast(
                            bc[:],
                            rec[0:1, s_slot * SPAN:(s_slot + 1) * SPAN])
                        nc.vector.tensor_mul(
                            zc[i][g * 64:(g + 1) * 64, :],
                            u_big[0:64, s_slot * SPAN:(s_slot + 1) * SPAN],
                            bc[:],
                        )

                # output projection for this span of s
                for st in range(4):
                    s0 = q0 + st * P
                    o_sb = op.tile([P, D], F32, tag="ost", name="osb")
                    for dsp in range(2):
                        o_ps = ps_sc.tile([P, SPAN], F32, tag="sc",
                                          name="ops")
                        for ch in range(2):
                            nc.tensor.matmul(
                                o_ps[:],
                                zc[ch][:, st * P:(st + 1) * P],
                                wo_sb[ch][:, dsp * SPAN:(dsp + 1) * SPAN],
                                start=(ch == 0),
                                stop=(ch == 1),
                            )
                        nc.vector.tensor_copy(
                            o_sb[:, dsp * SPAN:(dsp + 1) * SPAN], o_ps[:])
                    nc.sync.dma_start(out_d[s0:s0 + P, :], o_sb[:])

    nc.finalize()
    return nc


def kernel(resid, W_Q, W_K, W_V, W_out, b_out):
    global LAST_RESULTS, _CACHED_NC
    resid = np.asarray(resid, np.float32)
    W_Q = np.asarray(W_Q, np.float32)
    W_K = np.asarray(W_K, np.float32)
    W_V = np.asarray(W_V, np.float32)
    W_out = np.asarray(W_out, np.float32)
    b_out = np.asarray(b_out, np.float32)

    if _CACHED_NC is None:
        _CACHED_NC = _build_program()
    nc = _CACHED_NC

    bf = ml_dtypes.bfloat16
    residT = [np.ascontiguousarray(resid[b].T).astype(bf) for b in range(2)]
    in_maps = []
    for c in range(8):
        b, q = c // 4, c % 4
        # interleaved head order [h0, h2, h1, h3]: storage slot (g, i) holds
        # local head 2g+i -> qT[i]/zc[i] rows g*64 (see _build_program)
        heads = [4 * q, 4 * q + 2, 4 * q + 1, 4 * q + 3]
        groups = [2 * q, 2 * q + 1]
        in_maps.append({
            "resid_t": residT[b],
            "wq": np.ascontiguousarray(
                W_Q[:, heads, :].reshape(D, 256)).astype(bf),
            "wk": np.ascontiguousarray(
                W_K[:, groups, :].reshape(D, 128)).astype(bf),
            "wv": np.ascontiguousarray(
                W_V[:, groups, :].reshape(D, 128)).astype(bf),
            "wo": np.ascontiguousarray(
                W_out[:, heads, :].transpose(1, 0, 2).reshape(256, D)
            ).astype(bf),
        })

    res = run_bass_kernel_spmd(nc, in_maps, core_ids=list(range(8)))
    LAST_RESULTS = res

    out = np.zeros((2, S, D), np.float32)
    for c in range(8):
        out[c // 4] += res.results[c]["out"]
    out += b_out
    return out


# revision 6
# speedup vs baseline: 1.4647x; 1.1555x over previous
"""GQA attention kernel for Trainium2, 8 NeuronCores.

Problem: resid [2, 2048, 1024], 16 Q heads / 8 KV groups, d_head 64, causal,
out = softmax(QK^T/8 + causal) V -> W_out + b_out.

Sharding: tensor-parallel over (batch x kv-group-pairs). Core c handles
batch b = c // 4 and kv groups {2*(c%4), 2*(c%4)+1} = 4 Q heads. Each core
computes its heads' attention and a partial output projection; the host sums
the 4 partials per batch element and adds b_out.

Per-core dataflow (bf16 storage/matmuls, fp32 PSUM accumulation):
  - host passes resid[b].T in bf16; weights/resid arrive as one coalesced
    DMA each (many small DMAs serialize on the sync queue's ~0.6us issue)
  - Q^T [256, S] and K^T [128, S] projections (PSUM accum over 8 d-chunks)
  - V [S, 2x65] with a ones column appended per group -> the AV matmul
    produces sum-exp for free in output row 64
  - scores computed transposed: S^T[k, q] = K @ Q^T; causality via q-start
    offset and an upper-triangular multiplicative mask on diagonal tiles
  - per k-tile the two heads of a group land in one 2-bank PSUM tile so a
    single ACT exp instruction (2D access pattern) covers both
  - softmax without max-subtraction (scores are O(1) by construction;
    masked lanes are exactly zero after the mask multiply)
  - U^T[e, q] accumulates per HALF-span into a 2-bank PSUM tile (4 head
    slots x 256 q), double-buffered, so the normalize chain of one half
    overlaps the attention of the next; sum-exp row 64 is one contiguous
    [1, 1024] AP -> one Ln + one Exp(-x) per half
  - a pre-placed ACT table load of natural_log_exp_and_others serves both
    Exp and Ln (the default pass thrashes 17 table loads otherwise)
  - normalize: partition-broadcast of 1/sumexp (GpSimd), multiply (DVE)
  - out_partial[s, d] = z^T.T @ W_out_stack accumulated over 2 e-chunks
  - a filler queue weaves next-span Q/K/V projection groups and deferred
    normalize/output-projection work into the attention k-tile stream so
    the PE queue never blocks on the softmax denominators
"""

import sys

sys.path.insert(0, "/opt/trn_rl_repo")

from collections import deque

import ml_dtypes
import numpy as np

import concourse.bass as bass
import concourse.mybir as mybir
import concourse.tile as tile
from concourse import bacc
from concourse.bass_utils import run_bass_kernel_spmd
from concourse.hw_specs import get_activation_tables
from concourse.masks import make_upper_triangular

S = 2048          # seq len
D = 1024          # d_model
E = 64            # d_head
P = 128
NCHUNK = D // P   # 8 d_model chunks
SPAN = 512
HALF = 256
NSPAN = S // SPAN
NKT = S // P      # 16 k tiles
F32 = mybir.dt.float32
BF = mybir.dt.bfloat16
EXP = mybir.ActivationFunctionType.Exp
LN = mybir.ActivationFunctionType.Ln

LAST_RESULTS = None  # stashed BassKernelResults for the test harness
_CACHED_NC = None


def _build_program():
    nc = bacc.Bacc("TRN2", target_bir_lowering=False, debug=False)

    rT_d = nc.dram_tensor("resid_t", [D, S], BF, kind="ExternalInput")
    wq_d = nc.dram_tensor("wq", [D, 256], BF, kind="ExternalInput")
    wk_d = nc.dram_tensor("wk", [D, 128], BF, kind="ExternalInput")
    wv_d = nc.dram_tensor("wv", [D, 128], BF, kind="ExternalInput")
    wo_d = nc.dram_tensor("wo", [256, D], BF, kind="ExternalInput")
    out_d = nc.dram_tensor("out", [S, D], F32, kind="ExternalOutput")

    # one ACT table set serves Exp and Ln; pre-placing the load keeps the
    # fixpoint pass from alternating exp_and_others / natural_log sets
    tables = list(get_activation_tables(nc.m.arch).keys())
    nle_id = tables.index("natural_log_exp_and_others")

    with tile.TileContext(nc) as tc:
        with (
            tc.tile_pool(name="persist", bufs=1) as pp,
            tc.tile_pool(name="exp", bufs=6) as ep,
            tc.tile_pool(name="zt", bufs=2) as zp,
            tc.tile_pool(name="misc", bufs=2) as mp,
            tc.tile_pool(name="ostage", bufs=3) as op,
            tc.tile_pool(name="ps_u", bufs=2, space="PSUM") as ps_u,
            tc.tile_pool(name="ps_sc", bufs=2, space="PSUM") as ps_sc,
        ):
            nc.scalar.add_instruction(
                mybir.InstLoadActFuncSet(
                    name=nc.get_next_instruction_name(),
                    act_func_set_id=nle_id,
                )
            )

            # ---- coalesced weight loads (one DMA per weight tensor) ----
            wqb = pp.tile([P, NCHUNK * 256], BF, tag="wq")
            nc.sync.dma_start(
                wqb.rearrange("p (c n) -> p c n", c=NCHUNK),
                wq_d.rearrange("(c p) n -> p c n", p=P))
            wq_sb = [wqb[:, c * 256:(c + 1) * 256] for c in range(NCHUNK)]
            wkb = pp.tile([P, NCHUNK * 128], BF, tag="wk")
            nc.sync.dma_start(
                wkb.rearrange("p (c n) -> p c n", c=NCHUNK),
                wk_d.rearrange("(c p) n -> p c n", p=P))
            wk_sb = [wkb[:, c * 128:(c + 1) * 128] for c in range(NCHUNK)]
            wvb = pp.tile([P, NCHUNK * 128], BF, tag="wv")
            nc.sync.dma_start(
                wvb.rearrange("p (c n) -> p c n", c=NCHUNK),
                wv_d.rearrange("(c p) n -> p c n", p=P))
            wv_sb = [wvb[:, c * 128:(c + 1) * 128] for c in range(NCHUNK)]
            wob = pp.tile([P, 2 * D], BF, tag="wo")
            nc.sync.dma_start(
                wob.rearrange("p (c n) -> p c n", c=2),
                wo_d.rearrange("(c p) n -> p c n", p=P))
            wo_sb = [wob[:, c * D:(c + 1) * D] for c in range(2)]

            mask = pp.tile([P, P], BF, tag="mask")
            make_upper_triangular(nc, mask[:], val=1.0, diag=True)

            # residual: one 1MB DMA per span (all 8 d-chunks)
            rTb = pp.tile([P, NCHUNK * S], BF, tag="rt")
            rT = [rTb[:, c * S:(c + 1) * S] for c in range(NCHUNK)]
            rTb_v = rTb.rearrange("p (c m) -> p c m", c=NCHUNK)
            rTd_v = rT_d.rearrange("(c p) m -> p c m", p=P)
            for sp in range(NSPAN):
                nc.sync.dma_start(
                    rTb_v[:, :, sp * SPAN:(sp + 1) * SPAN],
                    rTd_v[:, :, sp * SPAN:(sp + 1) * SPAN])

            qT = [pp.tile([P, S], BF, tag=f"qt{e}", name=f"qt{e}")
                  for e in range(2)]
            kT = pp.tile([P, S], BF, tag="kt")
            vaug = [pp.tile([P, 130], BF, tag=f"va{k}", name=f"va{k}")
                    for k in range(NKT)]
            for k in range(NKT):
                nc.gpsimd.memset(vaug[k][:, 64:65], 1.0)
                nc.gpsimd.memset(vaug[k][:, 129:130], 1.0)

            # ---- emitters ----
            def q_proj(sp, eblk):
                acc = ps_sc.tile([P, SPAN], F32, tag="sc", name="qacc")
                for c in range(NCHUNK):
                    nc.tensor.matmul(
                        acc[:],
                        wq_sb[c][:, eblk * P:(eblk + 1) * P],
                        rT[c][:, sp * SPAN:(sp + 1) * SPAN],
                        start=(c == 0),
                        stop=(c == NCHUNK - 1),
                    )
                nc.vector.tensor_copy(
                    qT[eblk][:, sp * SPAN:(sp + 1) * SPAN], acc[:])

            def k_proj(sp):
                acc = ps_sc.tile([P, SPAN], F32, tag="sc", name="kacc")
                for c in range(NCHUNK):
                    nc.tensor.matmul(
                        acc[:],
                        wk_sb[c][:],
                        rT[c][:, sp * SPAN:(sp + 1) * SPAN],
                        start=(c == 0),
                        stop=(c == NCHUNK - 1),
                    )
                nc.vector.tensor_copy(kT[:, sp * SPAN:(sp + 1) * SPAN], acc[:])

            def v_proj(kt):
                va = vaug[kt]
                acc = ps_sc.tile([P, 128], F32, tag="sc", name="vacc")
                for c in range(NCHUNK):
                    nc.tensor.matmul(
                        acc[:, 0:128],
                        rT[c][:, kt * P:(kt + 1) * P],
                        wv_sb[c][:],
                        start=(c == 0),
                        stop=(c == NCHUNK - 1),
                    )
                nc.vector.tensor_copy(va[:, 0:64], acc[:, 0:64])
                nc.vector.tensor_copy(va[:, 65:129], acc[:, 64:128])

            proj_left = [0] * (NSPAN + 1)

            def proj_thunks(sp):
                def wrap(fn):
                    def run():
                        proj_left[sp] -= 1
                        fn()
                    return run
                th = [wrap(lambda e=e: q_proj(sp, e)) for e in range(2)]
                th.append(wrap(lambda: k_proj(sp)))
                th += [wrap(lambda kt=kt: v_proj(kt))
                       for kt in range(4 * sp, 4 * sp + 4)]
                proj_left[sp] += len(th)
                return th

            # u tile col layout: head slot s = 2g+i at cols s*HALF; row 64
            # of each slot is the sum-exp (ones column of vaug)
            def normalize(u_half, zc, hq0):
                # 1/x as exp(-ln x): ScalarE, one op per half-span; fp32
                # intermediates (bf16 ln would cost ~2% in the exp back off)
                lnt = mp.tile([1, 4 * HALF], F32, tag="ln", name="lnt")
                nc.scalar.activation(lnt[:], u_half[64:65, :], LN)
                rec = mp.tile([1, 4 * HALF], F32, tag="rec", name="rec")
                nc.scalar.activation(rec[:], lnt[:], EXP, scale=-1.0)
                for g in range(2):
                    for i in range(2):
                        s_slot = 2 * g + i
                        bc = mp.tile([64, HALF], F32, tag=f"bc{s_slot}",
                                     name="bc")
                        nc.gpsimd.partition_broadcast(
                            bc[:],
                            rec[0:1, s_slot * HALF:(s_slot + 1) * HALF])
                        nc.vector.tensor_mul(
                            zc[i][g * 64:(g + 1) * 64, :],
                            u_half[0:64,
                                   s_slot * HALF:(s_slot + 1) * HALF],
                            bc[:],
                        )

            def o_proj_st(zc, s0):
                # one 128-row block of the output projection + store
                o_sb = op.tile([P, D], F32, tag="ost", name="osb")
                for dsp in range(2):
                    o_ps = ps_sc.tile([P, SPAN], F32, tag="sc", name="ops")
                    st = (s0 // P) % 2
                    for ch in range(2):
                        nc.tensor.matmul(
                            o_ps[:],
                            zc[ch][:, st * P:(st + 1) * P],
                            wo_sb[ch][:, dsp * SPAN:(dsp + 1) * SPAN],
                            start=(ch == 0),
                            stop=(ch == 1),
                        )
                    nc.vector.tensor_copy(
                        o_sb[:, dsp * SPAN:(dsp + 1) * SPAN], o_ps[:])
                nc.sync.dma_start(out_d[s0:s0 + P, :], o_sb[:])

            fillq = deque()

            # span 0 projections run upfront (overlap the resid DMA tail)
            for th in proj_thunks(0):
                th()

            for sp in range(NSPAN):
                # this span's own projections MUST be emitted before its
                # first score matmuls read qT/kT/vaug (emission order is
                # dataflow order for a fixed SBUF slice)
                while proj_left[sp] > 0:
                    fillq.popleft()()
                if sp + 1 < NSPAN:
                    fillq.extend(proj_thunks(sp + 1))

                q0 = sp * SPAN
                nkt = (q0 + SPAN) // P   # k tiles touching this span
                hb = nkt - 2             # k tiles touching the low half
                u_halves = [
                    ps_u.tile([P, 4 * HALF], F32, tag="u", name=f"u{h}")
                    for h in range(2)
                ]
                zcs = [
                    [zp.tile([P, HALF], BF, tag=f"zt{h}{c}", name=f"z{c}")
                     for c in range(2)]
                    for h in range(2)
                ]

                # AV of k-tile kt is emitted after the scores+exp of kt+1,
                # hiding the ACT exp latency from the PE stream; each AV
                # batch entry splits over the two half-span u tiles
                def emit_av(batch):
                    for g, i, e_sb, kt_, off_, w_ in batch:
                        s_slot = 2 * g + i
                        for h in range(2):
                            lo = max(off_ - h * HALF, 0)
                            hi = min(off_ + w_ - h * HALF, HALF)
                            if lo >= hi:
                                continue
                            last_kt = hb - 1 if h == 0 else nkt - 1
                            # two head slots share each PSUM bank and
                            # start=True clears has_written for the WHOLE
                            # bank: only the bank's first slot may set it
                            # (the sibling's first write lands on cleared
                            # bits, which means overwrite -> still correct)
                            nc.tensor.matmul(
                                u_halves[h][0:65,
                                            s_slot * HALF + lo:
                                            s_slot * HALF + hi],
                                vaug[kt_][:, g * 65:(g + 1) * 65],
                                e_sb[:, i * SPAN + h * HALF + lo:
                                     i * SPAN + h * HALF + hi],
                                start=(kt_ == 0 and i == 0),
                                stop=(kt_ == last_kt),
                                skip_group_check=True,
                            )

                pending = []
                for kt in range(nkt):
                    k0 = kt * P
                    off = max(k0 - q0, 0)
                    w = SPAN - off
                    cur = []
                    for g in range(2):
                        # both i-heads of group g share one 2-bank PSUM
                        # tile -> a single exp instruction covers them
                        pair = ps_sc.tile([P, 2 * SPAN], F32, tag="sc",
                                          name=f"pair{g}")
                        for i in range(2):
                            nc.tensor.matmul(
                                pair[:, i * SPAN + off:i * SPAN + off + w],
                                kT[g * 64:(g + 1) * 64, k0:k0 + P],
                                qT[i][g * 64:(g + 1) * 64,
                                      q0 + off:q0 + off + w],
                                start=True,
                                stop=True,
                            )
                        e_sb = ep.tile([P, 2 * SPAN], BF, tag="e",
                                       name=f"e{g}")
                        pv = pair.rearrange("p (i w) -> p i w", i=2)
                        ev = e_sb.rearrange("p (i w) -> p i w", i=2)
                        nc.scalar.activation(
                            ev[:, :, off:off + w], pv[:, :, off:off + w],
                            EXP, scale=0.125,
                        )
                        if k0 >= q0:  # diagonal tile -> causal mask
                            mv = mask.unsqueeze(1).broadcast_to([P, 2, P])
                            nc.vector.tensor_mul(
                                ev[:, :, off:off + P],
                                ev[:, :, off:off + P],
                                mv,
                            )
                        cur.append((g, 0, e_sb, kt, off, w))
                        cur.append((g, 1, e_sb, kt, off, w))
                    emit_av(pending)
                    pending = cur
                    if kt == hb:
                        # low half's AV chain (k tiles 0..hb-1) just went
                        # out -> emit its normalize now, queue its O-proj
                        normalize(u_halves[0], zcs[0], q0)
                        fillq.append(
                            lambda z=zcs[0], s=q0: o_proj_st(z, s))
                        fillq.append(
                            lambda z=zcs[0], s=q0 + P: o_proj_st(z, s))
                    if fillq:
                        fillq.popleft()()
                emit_av(pending)

                hq1 = q0 + HALF
                if sp + 1 < NSPAN:
                    fillq.append(
                        lambda u=u_halves[1], z=zcs[1], s=hq1:
                        normalize(u, z, s))
                    fillq.append(lambda z=zcs[1], s=hq1: o_proj_st(z, s))
                    fillq.append(
                        lambda z=zcs[1], s=hq1 + P: o_proj_st(z, s))
                else:
                    normalize(u_halves[1], zcs[1], hq1)
                    while fillq:
                        fillq.popleft()()
                    o_proj_st(zcs[1], hq1)
                    o_proj_st(zcs[1], hq1 + P)

    nc.finalize()
    return nc


def kernel(resid, W_Q, W_K, W_V, W_out, b_out):
    global LAST_RESULTS, _CACHED_NC
    resid = np.asarray(resid, np.float32)
    W_Q = np.asarray(W_Q, np.float32)
    W_K = np.asarray(W_K, np.float32)
    W_V = np.asarray(W_V, np.float32)
    W_out = np.asarray(W_out, np.float32)
    b_out = np.asarray(b_out, np.float32)

    if _CACHED_NC is None:
        _CACHED_NC = _build_program()
    nc = _CACHED_NC

    bf = ml_dtypes.bfloat16
    residT = [np.ascontiguousarray(resid[b].T).astype(bf) for b in range(2)]
    in_maps = []
    for c in range(8):
        b, q = c // 4, c % 4
        # interleaved head order [h0, h2, h1, h3]: storage slot (g, i) holds
        # local head 2g+i -> qT[i]/zc[i] rows g*64 (see _build_program)
        heads = [4 * q, 4 * q + 2, 4 * q + 1, 4 * q + 3]
        groups = [2 * q, 2 * q + 1]
        in_maps.append({
            "resid_t": residT[b],
            "wq": np.ascontiguousarray(
                W_Q[:, heads, :].reshape(D, 256)).astype(bf),
            "wk": np.ascontiguousarray(
                W_K[:, groups, :].reshape(D, 128)).astype(bf),
            "wv": np.ascontiguousarray(
                W_V[:, groups, :].reshape(D, 128)).astype(bf),
            "wo": np.ascontiguousarray(
                W_out[:, heads, :].transpose(1, 0, 2).reshape(256, D)
            ).astype(bf),
        })

    res = run_bass_kernel_spmd(nc, in_maps, core_ids=list(range(8)))
    LAST_RESULTS = res

    out = np.zeros((2, S, D), np.float32)
    for c in range(8):
        out[c // 4] += res.results[c]["out"]
    out += b_out
    return out
